# revision 6
# baseline (speedup 1.0000x reference)
"""CapsuleMaxPooling Trainium2 kernel.

Problem: inp [B=32, C=32, H=64, W=64, D=8] f32, kernel_size k=2.
For each 2x2 spatial window pick the capsule vector (length D=8) with the
largest squared L2 norm (first-max tie-break) -> out [B, C, 32, 32, 8].

Strategy (fully data-parallel, shard B across 8 cores; per core the shard is
viewed as rows r=(b, c, hk) of 1024 contiguous floats = (dh, wk, dw, d).
Rows are assigned to partitions block-contiguously (partition p owns rows
r0+p*tb..r0+p*tb+tb-1 of a batch) so each partition's DMA side is one large
contiguous descriptor.

Engine balance: the d=8 norm reduction runs on the (otherwise idle) PE
array as 8 PSUM-accumulated identity matmuls -- out[p, w] += I.T @ sq[p,
(w, d=i)] -- which removes the whole reduce tree from DVE/ACT. Per-engine:
  - ACT: sq = x^2 (Square activation); evacuate norms PSUM->SBUF.
  - PE: 8 accumulated matmuls per batch (moving = sq d-slice, N=tb*128
    <= 512 = fp32 moving limit = one PSUM bank).
  - DVE: base copy of candidate D into the output tile (tensor_copy, 2x_2p
    fast mode), the 6-op mask tournament (M = max of 4 norms, wX = nX >= M)
    and the 3 copy_predicated overwrites (C, B, A; order gives first-max).
  - GpSimd: only the one-time identity build (it cannot access PSUM, and
    its tensor ops are ~2ns/elem and contend with DVE's port pair).
  - copy_predicated wants an integer mask: int32 bitcast view of the f32
    0.0/1.0 mask (1.0f = 0x3F800000 != 0) broadcast over d via stride-0.
  - HWDGE (nc.sync) DMAs. The batch schedule starts and ends with small
    batches to shorten pipeline ramp-in/ramp-out.
"""

import numpy as np

try:
    import concourse.bass as bass
except ImportError:  # pragma: no cover
    import sys

    sys.path.insert(0, "/opt/trn_rl_repo")
    import concourse.bass as bass

from concourse import bacc, mybir
from concourse.bass_utils import run_bass_kernel_spmd
from concourse.masks import make_identity
from concourse.tile import TileContext

P = 128
N_CORES = 8
ROW_W = 1024  # (dh=2) * (wk=32) * (dw=2) * (d=8)
OUT_W = 256  # (wk=32) * (d=8)
# row-tiles per batch; sums to R // P (= 32).
DEFAULT_SCHED = (1, 1, 2, 2, 4, 4, 4, 4, 4, 4, 2)


def _bcs(w, q0, qn, n):
    """Slice mask tile w [P, GTB, 32] rows [q0:q0+qn], viewed as int32
    [P, qn, 32, n] via a bitcast + stride-0 inner dim (copy_predicated
    wants an integer mask; 1.0f = 0x3F800000 != 0)."""
    a = w[:, q0 : q0 + qn].bitcast(mybir.dt.int32)
    return bass.AP(tensor=a.tensor, offset=a.offset, ap=[*a.ap, [0, n]])


def build_nc(R=4096, sched=DEFAULT_SCHED, GM=2):
    """Build the per-core Bass program. R = rows (b,c,hk) per core."""
    f32 = mybir.dt.float32
    nc = bacc.Bacc(None, target_bir_lowering=False)
    x = nc.dram_tensor("x", [R, ROW_W], f32, kind="ExternalInput")
    y = nc.dram_tensor("y", [R, OUT_W], f32, kind="ExternalOutput")
    assert sum(sched) * P == R
    # group consecutive batches for the mask stage (amortizes small-op cost)
    groups = [list(sched[i : i + GM]) for i in range(0, len(sched), GM)]

    with TileContext(nc) as tc:
        with (
            tc.tile_pool(name="constp", bufs=1) as constp,
            tc.tile_pool(name="xp", bufs=6) as xp,
            tc.tile_pool(name="sqp", bufs=3) as sqp,
            tc.tile_pool(name="normp", bufs=2) as normp,
            tc.tile_pool(name="maskp", bufs=2) as maskp,
            tc.tile_pool(name="outp", bufs=4) as outp,
            tc.psum_pool(name="npp", bufs=4) as npp,
        ):
            ident = constp.tile([P, P], f32, tag="ident")
            make_identity(nc, ident)

            tile0 = 0
            for grp in groups:
                gtb = sum(grp)
                norms = normp.tile([P, gtb, 128], f32, tag="norms")
                xts = []
                ots = []
                qoff = [0]
                for tb in grp:
                    r0 = tile0 * P
                    xt = xp.tile([P, tb, ROW_W], f32, tag="xt")
                    xts.append(xt)
                    nc.sync.dma_start(
                        out=xt,
                        in_=x[r0 : r0 + tb * P, :].rearrange(
                            "(p j) c -> p j c", j=tb
                        ),
                    )
                    sq = sqp.tile([P, tb, ROW_W], f32, tag="sq")
                    nc.scalar.square(sq, xt)
                    # d=8 reduction on the PE array: 8 accumulated identity
                    # matmuls into one PSUM bank (norms for this batch).
                    np_t = npp.tile([P, tb, 128], f32, tag="np")
                    sqv = sq.rearrange(
                        "p j (dh wk dw dd) -> p j dh wk dw dd", dh=2, dw=2, dd=8
                    )
                    npv = np_t.rearrange(
                        "p j (dh wk dw) -> p j dh wk dw", dh=2, dw=2
                    )
                    for di in range(8):
                        nc.tensor.matmul(
                            npv,
                            ident,
                            sqv[:, :, :, :, :, di],
                            start=(di == 0),
                            stop=(di == 7),
                        )
                    # evacuate norms PSUM -> SBUF (ACT)
                    q0 = qoff[-1]
                    nc.scalar.copy(norms[:, q0 : q0 + tb], np_t)

                    ot = outp.tile([P, tb, 32, 8], f32, tag="ot")
                    ots.append(ot)
                    xr = xt.rearrange(
                        "p j (dh wk dw d) -> p j dh wk dw d", dh=2, dw=2, d=8
                    )
                    # base copy of candidate D on DVE (2x_2p fast mode)
                    nc.vector.tensor_copy(ot, xr[:, :, 1, :, 1, :])
                    qoff.append(q0 + tb)
                    tile0 += tb

                # 6-op tournament on the whole group's norms (DVE)
                nr = norms.rearrange("p j (dh wk dw) -> p j dh wk dw", dh=2, dw=2)
                nA = nr[:, :, 0, :, 0]
                nB = nr[:, :, 0, :, 1]
                nC = nr[:, :, 1, :, 0]
                nD = nr[:, :, 1, :, 1]

                h1 = maskp.tile([P, gtb, 32], f32, tag="h1")
                nc.vector.tensor_tensor(h1, nA, nB, op=mybir.AluOpType.max)
                h2 = maskp.tile([P, gtb, 32], f32, tag="h2")
                nc.vector.tensor_tensor(h2, nC, nD, op=mybir.AluOpType.max)
                M = maskp.tile([P, gtb, 32], f32, tag="M")
                nc.vector.tensor_tensor(M, h1, h2, op=mybir.AluOpType.max)
                wA = maskp.tile([P, gtb, 32], f32, tag="wA")
                nc.vector.tensor_tensor(wA, nA, M, op=mybir.AluOpType.is_ge)
                wB = maskp.tile([P, gtb, 32], f32, tag="wB")
                nc.vector.tensor_tensor(wB, nB, M, op=mybir.AluOpType.is_ge)
                wC = maskp.tile([P, gtb, 32], f32, tag="wC")
                nc.vector.tensor_tensor(wC, nC, M, op=mybir.AluOpType.is_ge)

                tile1 = tile0 - gtb
                for qi, tb in enumerate(grp):
                    r0 = tile1 * P
                    xt = xts[qi]
                    ot = ots[qi]
                    xr = xt.rearrange(
                        "p j (dh wk dw d) -> p j dh wk dw d", dh=2, dw=2, d=8
                    )
                    Av = xr[:, :, 0, :, 0, :]
                    Bv = xr[:, :, 0, :, 1, :]
                    Cv = xr[:, :, 1, :, 0, :]
                    q0 = qoff[qi]
                    nc.vector.copy_predicated(ot, _bcs(wC, q0, tb, 8), Cv)
                    nc.vector.copy_predicated(ot, _bcs(wB, q0, tb, 8), Bv)
                    nc.vector.copy_predicated(ot, _bcs(wA, q0, tb, 8), Av)

                    nc.sync.dma_start(
                        out=y[r0 : r0 + tb * P, :].rearrange(
                            "(p j) c -> p j c", j=tb
                        ),
                        in_=ot.rearrange("p j w d -> p j (w d)"),
                    )
                    tile1 += tb
    nc.compile()
    return nc


_NC_CACHE = {}


def _get_nc(R):
    if R not in _NC_CACHE:
        _NC_CACHE[R] = build_nc(R)
    return _NC_CACHE[R]


def kernel(inp, kernel_size):
    inp = np.asarray(inp)
    k = int(np.asarray(kernel_size))
    assert k == 2, f"kernel hardcoded for kernel_size=2, got {k}"
    B, C, H, W, D = inp.shape
    assert (B, C, H, W, D) == (32, 32, 64, 64, 8), inp.shape
    Hk, Wk = H // k, W // k

    bs = B // N_CORES  # 4 batches per core
    R = bs * C * Hk  # 4096 rows per core
    nc = _get_nc(R)

    in_maps = []
    for c in range(N_CORES):
        shard = np.ascontiguousarray(inp[c * bs : (c + 1) * bs]).reshape(R, ROW_W)
        in_maps.append({"x": shard})

    res = run_bass_kernel_spmd(nc, in_maps, list(range(N_CORES)))
    out = np.concatenate(
        [r["y"].reshape(bs, C, Hk, Wk, D) for r in res.results], axis=0
    )
    return out


# revision 12
# speedup vs baseline: 1.6168x; 1.6168x over previous
"""CapsuleMaxPooling Trainium2 kernel.

Problem: inp [B=32, C=32, H=64, W=64, D=8] f32, kernel_size k=2.
For each 2x2 spatial window pick the capsule vector (length D=8) with the
largest squared L2 norm (first-max tie-break) -> out [B, C, 32, 32, 8].

Strategy (fully data-parallel, shard B across 8 cores; per core the shard is
viewed as rows r=(b, c, hk) of 1024 contiguous floats = (dh, wk, dw, d).
Rows are assigned to partitions block-contiguously (partition p owns rows
r0+p*tb..r0+p*tb+tb-1 of a batch) so each partition's DMA side is one large
contiguous descriptor.

Engine balance: the d=8 norm reduction runs on the (otherwise idle) PE
array as 8 PSUM-accumulated identity matmuls -- out[p, w] += I.T @ sq[p,
(w, d=i)] -- which removes the whole reduce tree from DVE/ACT. Per-engine:
  - ACT: sq = x^2 (Square activation); evacuate norms PSUM->SBUF.
  - PE: 8 accumulated matmuls per batch (moving = sq d-slice, N=tb*128
    <= 512 = fp32 moving limit = one PSUM bank).
  - DVE: base copy of candidate D into the output tile (tensor_copy, 2x_2p
    fast mode), the 6-op mask tournament (M = max of 4 norms, wX = nX >= M)
    and the 3 copy_predicated overwrites (C, B, A; order gives first-max).
  - GpSimd: only the one-time identity build (it cannot access PSUM, and
    its tensor ops are ~2ns/elem and contend with DVE's port pair).
  - copy_predicated wants an integer mask: int32 bitcast view of the f32
    0.0/1.0 mask (1.0f = 0x3F800000 != 0) broadcast over d via stride-0.
  - HWDGE (nc.sync) DMAs. The batch schedule starts and ends with small
    batches to shorten pipeline ramp-in/ramp-out.
"""

import numpy as np

try:
    import concourse.bass as bass
except ImportError:  # pragma: no cover
    import sys

    sys.path.insert(0, "/opt/trn_rl_repo")
    import concourse.bass as bass

from concourse import bacc, mybir
from concourse.bass_utils import run_bass_kernel_spmd
from concourse.masks import make_identity
from concourse.tile import TileContext

P = 128
N_CORES = 8
ROW_W = 1024  # (dh=2) * (wk=32) * (dw=2) * (d=8)
OUT_W = 256  # (wk=32) * (d=8)
# row-tiles per batch; sums to R // P (= 32).
DEFAULT_SCHED = (1, 1, 2, 2, 4, 4, 4, 4, 4, 4, 2)


def _bcs(w, q0, qn, n):
    """Slice mask tile w [P, GTB, 32] rows [q0:q0+qn], viewed as int32
    [P, qn, 32, n] via a bitcast + stride-0 inner dim (copy_predicated
    wants an integer mask; 1.0f = 0x3F800000 != 0)."""
    a = w[:, q0 : q0 + qn].bitcast(mybir.dt.int32)
    return bass.AP(tensor=a.tensor, offset=a.offset, ap=[*a.ap, [0, n]])


def build_nc(R=4096, sched=DEFAULT_SCHED, GM=2):
    """Build the per-core Bass program. R = rows (b,c,hk) per core."""
    f32 = mybir.dt.float32
    nc = bacc.Bacc(None, target_bir_lowering=False)
    x = nc.dram_tensor("x", [R, ROW_W], f32, kind="ExternalInput")
    y = nc.dram_tensor("y", [R, OUT_W], f32, kind="ExternalOutput")
    assert sum(sched) * P == R
    # group consecutive batches for the mask stage (amortizes small-op cost)
    groups = [list(sched[i : i + GM]) for i in range(0, len(sched), GM)]

    with TileContext(nc) as tc:
        with (
            tc.tile_pool(name="constp", bufs=1) as constp,
            tc.tile_pool(name="xp", bufs=6) as xp,
            tc.tile_pool(name="sqp", bufs=3) as sqp,
            tc.tile_pool(name="normp", bufs=2) as normp,
            tc.tile_pool(name="maskp", bufs=2) as maskp,
            tc.tile_pool(name="outp", bufs=4) as outp,
            tc.psum_pool(name="npp", bufs=4) as npp,
        ):
            # float32r mode: single-pass fp32 matmul (4x faster than the
            # hi/lo-decomposed float32 path when the moving dim is >= 256).
            # The verifier requires fp32r matmul operands to be produced
            # pre-rounded, so sq is fp32r-typed and the identity is copied
            # (rounded) into an fp32r tile by the ACT engine.
            f32r = mybir.dt.float32r
            ident0 = constp.tile([P, P], f32, tag="ident0")
            make_identity(nc, ident0)
            ident = constp.tile([P, P], f32r, tag="ident")
            nc.scalar.copy(ident, ident0)

            tile0 = 0
            for grp in groups:
                gtb = sum(grp)
                norms = normp.tile([P, gtb, 128], f32, tag="norms")
                xts = []
                ots = []
                qoff = [0]
                for tb in grp:
                    r0 = tile0 * P
                    xt = xp.tile([P, tb, ROW_W], f32, tag="xt")
                    xts.append(xt)
                    nc.sync.dma_start(
                        out=xt,
                        in_=x[r0 : r0 + tb * P, :].rearrange(
                            "(p j) c -> p j c", j=tb
                        ),
                    )
                    sq = sqp.tile([P, tb, ROW_W], f32r, tag="sq")
                    nc.scalar.square(sq, xt)
                    # d=8 reduction on the PE array: 8 accumulated identity
                    # matmuls into one PSUM bank (norms for this batch).
                    np_t = npp.tile([P, tb, 128], f32, tag="np")
                    sqv = sq.rearrange(
                        "p j (dh wk dw dd) -> p j dh wk dw dd", dh=2, dw=2, dd=8
                    )
                    npv = np_t.rearrange(
                        "p j (dh wk dw) -> p j dh wk dw", dh=2, dw=2
                    )
                    for di in range(8):
                        nc.tensor.matmul(
                            npv,
                            ident,
                            sqv[:, :, :, :, :, di],
                            start=(di == 0),
                            stop=(di == 7),
                        )
                    # evacuate norms PSUM -> SBUF (ACT)
                    q0 = qoff[-1]
                    nc.scalar.copy(norms[:, q0 : q0 + tb], np_t)

                    ot = outp.tile([P, tb, 32, 8], f32, tag="ot")
                    ots.append(ot)
                    xr = xt.rearrange(
                        "p j (dh wk dw d) -> p j dh wk dw d", dh=2, dw=2, d=8
                    )
                    # base copy of candidate D on DVE (2x_2p fast mode)
                    nc.vector.tensor_copy(ot, xr[:, :, 1, :, 1, :])
                    qoff.append(q0 + tb)
                    tile0 += tb

                # 6-op tournament on the whole group's norms (DVE)
                nr = norms.rearrange("p j (dh wk dw) -> p j dh wk dw", dh=2, dw=2)
                nA = nr[:, :, 0, :, 0]
                nB = nr[:, :, 0, :, 1]
                nC = nr[:, :, 1, :, 0]
                nD = nr[:, :, 1, :, 1]

                h1 = maskp.tile([P, gtb, 32], f32, tag="h1")
                nc.vector.tensor_tensor(h1, nA, nB, op=mybir.AluOpType.max)
                h2 = maskp.tile([P, gtb, 32], f32, tag="h2")
                nc.vector.tensor_tensor(h2, nC, nD, op=mybir.AluOpType.max)
                M = maskp.tile([P, gtb, 32], f32, tag="M")
                nc.vector.tensor_tensor(M, h1, h2, op=mybir.AluOpType.max)
                wA = maskp.tile([P, gtb, 32], f32, tag="wA")
                nc.vector.tensor_tensor(wA, nA, M, op=mybir.AluOpType.is_ge)
                wB = maskp.tile([P, gtb, 32], f32, tag="wB")
                nc.vector.tensor_tensor(wB, nB, M, op=mybir.AluOpType.is_ge)
                wC = maskp.tile([P, gtb, 32], f32, tag="wC")
                nc.vector.tensor_tensor(wC, nC, M, op=mybir.AluOpType.is_ge)

                tile1 = tile0 - gtb
                for qi, tb in enumerate(grp):
                    r0 = tile1 * P
                    xt = xts[qi]
                    ot = ots[qi]
                    xr = xt.rearrange(
                        "p j (dh wk dw d) -> p j dh wk dw d", dh=2, dw=2, d=8
                    )
                    Av = xr[:, :, 0, :, 0, :]
                    Bv = xr[:, :, 0, :, 1, :]
                    Cv = xr[:, :, 1, :, 0, :]
                    q0 = qoff[qi]
                    nc.vector.copy_predicated(ot, _bcs(wC, q0, tb, 8), Cv)
                    nc.vector.copy_predicated(ot, _bcs(wB, q0, tb, 8), Bv)
                    nc.vector.copy_predicated(ot, _bcs(wA, q0, tb, 8), Av)

                    nc.sync.dma_start(
                        out=y[r0 : r0 + tb * P, :].rearrange(
                            "(p j) c -> p j c", j=tb
                        ),
                        in_=ot.rearrange("p j w d -> p j (w d)"),
                    )
                    tile1 += tb
    nc.compile()
    return nc


_NC_CACHE = {}


def _get_nc(R):
    if R not in _NC_CACHE:
        _NC_CACHE[R] = build_nc(R)
    return _NC_CACHE[R]


def kernel(inp, kernel_size):
    inp = np.asarray(inp)
    k = int(np.asarray(kernel_size))
    assert k == 2, f"kernel hardcoded for kernel_size=2, got {k}"
    B, C, H, W, D = inp.shape
    assert (B, C, H, W, D) == (32, 32, 64, 64, 8), inp.shape
    Hk, Wk = H // k, W // k

    bs = B // N_CORES  # 4 batches per core
    R = bs * C * Hk  # 4096 rows per core
    nc = _get_nc(R)

    in_maps = []
    for c in range(N_CORES):
        shard = np.ascontiguousarray(inp[c * bs : (c + 1) * bs]).reshape(R, ROW_W)
        in_maps.append({"x": shard})

    res = run_bass_kernel_spmd(nc, in_maps, list(range(N_CORES)))
    out = np.concatenate(
        [r["y"].reshape(bs, C, Hk, Wk, D) for r in res.results], axis=0
    )
    return out
